# revision 1
# baseline (speedup 1.0000x reference)
"""Trainium2 Bass kernel for CombinedRankingLoss (BCE + pairwise margin ranking).

Full inputs: logits/labels/weights [64, 1024, 1] f32. Output: scalar f32.

Data-parallel over batch: 8 cores x 8 batches. The pairwise term
    T_b = sum_{i in pos} sum_{j in neg} relu((v_j + M) - v_i)
is computed per batch with the candidates PERMUTED on host (the loss is
invariant to per-batch candidate order):
  - a side (i): pos-compacted; first KA=512 rows per batch (4 chunks of 128),
    rows beyond 512 packed into NOVF=2 SHARED overflow chunks (mixed batches,
    attributed per-partition on host); sent negated, pads -BIG
  - b side (j): neg-compacted (v+M), padded to KB=576 with -BIG, sent as a
    bf16 hi/lo pair whose sum reconstructs f32 to ~1e-4
Masked pads contribute exactly 0 through the relu; any batch that exceeds the
padding (prob ~1e-15) is recomputed exactly on host with its device
contribution masked to 0.

Per block (8 batch blocks + 2 overflow blocks) on device:
  - PE: one [16,128] bf16 selector matmul broadcasts (hi + lo) b-rows to a
    [128, KB] f32 PSUM tile (per-partition batch selection for overflow).
  - chunk ops: fused compare+row-reduce, ONE instruction per 128x576 tile,
    split ~17:17 across ScalarE and VectorE (both read PSUM directly):
      ScalarE: activation(Relu, bias=-a_col, accum_out)  -> sum_j relu(b_j-a_i)
      VectorE: tensor_scalar(op0=max a_col, op1=add-as-REDUCER, accum_out)
               -> sum_j max(b_j, a_i); host subtracts KB*a_i (exact identity
               sum_j max(b_j,a)-KB*a = sum_j relu(b_j-a))
  - BCE partials on flat [128,64] tiles (softplus = ln(1+exp), one ACT table
    set shared with Relu).
  - accumulator tiles [128,48] DMA'd out; host does the partition reduction.
Host: per-batch counts/means/valid handling + final scalar (f64).
"""
import sys
import numpy as np

sys.path.insert(0, "/opt/trn_rl_repo")

B, N = 64, 1024
N_CORES = 8
BLOC = B // N_CORES          # batches per core
KA = 512                     # fixed pos rows per batch; 4 chunks of 128
KB = 576                     # padded neg-count (j side, free dim); overflow
                             # falls back to exact host recompute per batch
NCH = KA // 128              # fixed chunks per batch (4)
NOVF = 2                     # shared overflow chunks (pos rows beyond 512,
                             # mixed batches; host attributes per partition)
MARGIN = 0.5
BIG = 16.0                   # mask sentinel; dominates |v|+margin (|v|<~5.5)
NACC = NCH * BLOC + NOVF     # 34 accumulator columns
NOUT = 48
SELW = N + NOVF * 128        # selector width: 8 batch blocks + overflow blocks

_CACHE = {}


def _patch_bass(bass):
    """Split multi-wait instructions (old walrus TPB_CTRL takes 1 wait)."""
    import json as _json
    if getattr(bass.Bass, "_wait_split_patched", False):
        return
    _orig = bass.Bass.to_json_bytes

    def _split(bir, limit=1):
        m = _json.loads(bir)
        for fn in m["functions"]:
            for bb in fn["blocks"]:
                out = []
                for i in bb.get("instructions", []):
                    si = i.get("sync_info") or {}
                    ow = si.get("on_wait") or []
                    if len(ow) > limit:
                        extra, keep = ow[:-limit], ow[-limit:]
                        for k, w in enumerate(extra):
                            out.append({
                                "debug": i.get("debug"), "engine": i["engine"],
                                "ins": [], "outs": [],
                                "name": i["name"] + f"_ws{k}",
                                "opcode": "NoOp",
                                "sync_info": {"on_wait": [w]},
                            })
                        si = dict(si)
                        si["on_wait"] = keep
                        i = dict(i)
                        i["sync_info"] = si
                    out.append(i)
                bb["instructions"] = out
        return _json.dumps(m).encode()

    bass.Bass.to_json_bytes = lambda self: _split(_orig(self))
    bass.Bass._wait_split_patched = True


def _engine_for(k):
    # 18:16 DVE:ACT split over 32 fixed + 2 overflow chunk ops (ACT pays a
    # ~182ns accumulator-read per op and also owns the BCE transcendentals)
    if k >= NCH * BLOC:                      # overflow chunks
        return "dve"
    b, c = divmod(k, NCH)
    return "dve" if (c + b) % 2 == 0 else "act"


def _build(bass, tile, mybir):
    f32 = mybir.dt.float32
    bf16 = mybir.dt.bfloat16
    Alu = mybir.AluOpType
    Act = mybir.ActivationFunctionType

    nc = bass.Bass()
    na_d = nc.declare_dram_parameter("na", [BLOC, KA], f32, isOutput=False)
    b2_d = nc.declare_dram_parameter("b2", [16, KB], bf16, isOutput=False)
    v_d = nc.declare_dram_parameter("v", [128, 64], f32, isOutput=False)
    y_d = nc.declare_dram_parameter("y", [128, 64], f32, isOutput=False)
    w_d = nc.declare_dram_parameter("w", [128, 64], f32, isOutput=False)
    sel_d = nc.declare_dram_parameter("sel", [16, SELW], bf16, isOutput=False)
    naov_d = nc.declare_dram_parameter("naov", [128, 2 * NOVF], f32, isOutput=False)
    id8_d = nc.declare_dram_parameter("id8", [8, 8], f32, isOutput=False)
    outd_d = nc.declare_dram_parameter("outd", [128, NOUT], f32, isOutput=True)
    outa_d = nc.declare_dram_parameter("outa", [128, NOUT], f32, isOutput=True)

    with tile.TileContext(nc) as tc:
        with (
            tc.tile_pool(name="const", bufs=1) as const,
            tc.tile_pool(name="work", bufs=2) as work,
            tc.tile_pool(name="psum", bufs=3, space="PSUM") as psum,
            tc.tile_pool(name="psum1", bufs=1, space="PSUM") as psum1,
        ):
            # ---------- loads (transpose inputs first: a-cols prep gates the
            # first chunk ops; broadcast matmuls overlap the transposes) -----
            na_rows = const.tile([BLOC, KA], f32)
            nc.sync.dma_start(out=na_rows[:], in_=na_d[:])
            ident8 = const.tile([8, 8], f32)
            nc.sync.dma_start(out=ident8[:], in_=id8_d[:])
            b2 = const.tile([16, KB], bf16)
            nc.sync.dma_start(out=b2[:], in_=b2_d[:])
            sel16 = const.tile([16, SELW], bf16)
            nc.sync.dma_start(out=sel16[:], in_=sel_d[:])
            # overflow a-columns, column layout direct from host: [-a | +a]
            naov = const.tile([128, 2 * NOVF], f32)
            nc.sync.dma_start(out=naov[:], in_=naov_d[:])
            v_flat = const.tile([128, 64], f32)
            y_flat = const.tile([128, 64], f32)
            w_flat = const.tile([128, 64], f32)
            nc.sync.dma_start(out=v_flat[:], in_=v_d[:])
            nc.sync.dma_start(out=y_flat[:], in_=y_d[:])
            nc.sync.dma_start(out=w_flat[:], in_=w_d[:])

            # accumulators
            dve_acc = const.tile([128, NOUT], f32)
            act_acc = const.tile([128, NOUT], f32)
            nc.vector.memset(dve_acc[:], 0.0)
            nc.vector.memset(act_acc[:], 0.0)

            # ---------- a-columns via PE transpose ----------
            tp = psum1.tile([128, NCH * 8], f32)
            for c in range(NCH):
                nc.tensor.transpose(tp[:, c * 8:(c + 1) * 8],
                                    na_rows[:, c * 128:(c + 1) * 128], ident8[:])
            na_cols = const.tile([128, NCH * 8], f32)   # -a  (ACT bias)
            nc.vector.tensor_copy(out=na_cols[:], in_=tp[:])
            a_cols = const.tile([128, NCH * 8], f32)    # +a  (DVE max operand)
            nc.vector.tensor_scalar(out=a_cols[:], in0=tp[:], scalar1=-1.0,
                                    scalar2=None, op0=Alu.mult)

            # BCE (flat [128,64]) — emitted mid-loop (see emit_bce below) so
            # the ACT queue leads with Relu chunk ops
            def emit_bce():
                sp = work.tile([128, 64], f32)
                nc.scalar.activation(out=sp[:], in_=v_flat[:], func=Act.Exp)
                nc.vector.tensor_scalar(out=sp[:], in0=sp[:], scalar1=1.0,
                                        scalar2=None, op0=Alu.add)
                nc.scalar.activation(out=sp[:], in_=sp[:], func=Act.Ln)
                xy = work.tile([128, 64], f32)
                nc.vector.tensor_tensor(out=xy[:], in0=v_flat[:], in1=y_flat[:],
                                        op=Alu.mult)
                nc.vector.tensor_tensor(out=xy[:], in0=sp[:], in1=xy[:],
                                        op=Alu.subtract)
                bce_scr = work.tile([128, 64], f32)
                nc.vector.scalar_tensor_tensor(
                    out=bce_scr[:], in0=xy[:], scalar=1.0, op0=Alu.mult,
                    op1=Alu.mult, in1=w_flat[:],
                    accum_out=dve_acc[:, NACC:NACC + 1])

            # ---------- main pairwise loop (software-pipelined emission) ----
            def emit_mm(b):
                bcp = psum.tile([128, KB], f32, tag="bcp")    # pool pads to banks
                lhsT = sel16[:, b * 128:(b + 1) * 128]
                nc.tensor.matmul(bcp[:, 0:512], lhsT, b2[:, 0:512],
                                 start=True, stop=True)
                nc.tensor.matmul(bcp[:, 512:KB], lhsT, b2[:, 512:KB],
                                 start=True, stop=True)
                return bcp

            def emit_chunk(k, bcp, nac, ac):
                col = k
                if _engine_for(k) == "act":
                    scr_act = work.tile([128, KB], f32, tag="scr_act")
                    nc.scalar.activation(
                        out=scr_act[:], in_=bcp[:], func=Act.Relu,
                        bias=nac, scale=1.0,
                        accum_out=act_acc[:, col:col + 1])
                else:
                    scr_dve = work.tile([128, KB], f32, tag="scr_dve")
                    nc.vector.tensor_scalar(
                        out=scr_dve[:], in0=bcp[:], scalar1=ac, scalar2=None,
                        op0=Alu.max, op1=Alu.add,
                        accum_out=dve_acc[:, col:col + 1])

            # block index sequence: 8 batch blocks then NOVF overflow blocks;
            # selector column block i covers sel16[:, i*128:(i+1)*128]
            bcp_q = [emit_mm(b) for b in range(3)]
            for b in range(BLOC + NOVF):
                bcp = bcp_q[b % 3]
                if b < BLOC:
                    ids = list(range(NCH))
                    ids.sort(key=lambda c: _engine_for(b * NCH + c) != "dve")
                    for c in ids:
                        k = b * NCH + c
                        cidx = c * 8 + b
                        emit_chunk(k, bcp, na_cols[:, cidx:cidx + 1],
                                   a_cols[:, cidx:cidx + 1])
                else:
                    j = b - BLOC
                    k = NCH * BLOC + j
                    emit_chunk(k, bcp, naov[:, j:j + 1],
                               naov[:, NOVF + j:NOVF + j + 1])
                if b == 1:
                    emit_bce()
                if b + 3 < BLOC + NOVF:
                    bcp_q[(b + 3) % 3] = emit_mm(b + 3)

            # ---------- results out (host does the partition reduction) ----
            nc.sync.dma_start(out=outd_d[:], in_=dve_acc[:])
            nc.sync.dma_start(out=outa_d[:], in_=act_acc[:])

    return nc


def _get_nc():
    if "nc" not in _CACHE:
        import concourse.bass as bass
        import concourse.tile as tile
        from concourse import mybir
        _patch_bass(bass)
        _CACHE["nc"] = _build(bass, tile, mybir)
    return _CACHE["nc"]


def _prep_core(v, y):
    """Compact one core's batches. Returns na [BLOC,KA] f32 (negated a, pads
    -BIG), b2 [16,KB] bf16 (hi/lo), naov [128,2*NOVF] f32 ([-a | +a] overflow
    columns), ovf_batch [NOVF*128] int (partition -> local batch, -1 pad),
    ovf_a [NOVF*128] f64, a_pad [BLOC,KA] f64, overflow list (host fallback)."""
    import ml_dtypes
    na = np.full((BLOC, KA), -BIG, dtype=np.float32)
    b_pad = np.full((BLOC, KB), -BIG, dtype=np.float32)
    ovf_batch = np.full(NOVF * 128, -1, dtype=np.int64)
    ovf_a = np.full(NOVF * 128, BIG, dtype=np.float64)
    overflow = []
    ptr = 0
    for r in range(BLOC):
        pos = v[r][y[r] == 1.0]
        neg = v[r][y[r] == 0.0] + np.float32(MARGIN)
        extra = len(pos) - KA
        if len(neg) > KB or (extra > 0 and ptr + extra > NOVF * 128):
            overflow.append(r)   # leave na row at -BIG => device contributes 0
            continue
        npos = min(len(pos), KA)
        na[r, :npos] = -pos[:npos]
        if extra > 0:
            ovf_a[ptr:ptr + extra] = pos[KA:].astype(np.float64)
            ovf_batch[ptr:ptr + extra] = r
            ptr += extra
        b_pad[r, :len(neg)] = neg
    naov = np.zeros((128, 2 * NOVF), dtype=np.float32)
    for j in range(NOVF):
        seg = ovf_a[j * 128:(j + 1) * 128].astype(np.float32)
        naov[:, j] = -seg
        naov[:, NOVF + j] = seg
    hi = b_pad.astype(ml_dtypes.bfloat16)
    lo = (b_pad - hi.astype(np.float32)).astype(ml_dtypes.bfloat16)
    b2 = np.concatenate([hi, lo], axis=0)           # [16, KB]
    return na, b2, naov, ovf_batch, ovf_a, -na.astype(np.float64), overflow


def make_in_maps(v, y, w):
    import ml_dtypes
    id8 = np.eye(8, dtype=np.float32)
    in_maps, a_pads, overflows, ovf_batches, ovf_as = [], [], [], [], []
    for c in range(N_CORES):
        sl = slice(c * BLOC, (c + 1) * BLOC)
        na, b2, naov, ovf_batch, ovf_a, a_pad, ovf = _prep_core(v[sl], y[sl])
        a_pads.append(a_pad)
        overflows.append(ovf)
        ovf_batches.append(ovf_batch)
        ovf_as.append(ovf_a)
        sel = np.zeros((16, SELW), dtype=np.float32)
        for b in range(BLOC):
            sel[b, b * 128:(b + 1) * 128] = 1.0
            sel[b + 8, b * 128:(b + 1) * 128] = 1.0
        for j in range(NOVF):
            for p in range(128):
                bb = ovf_batch[j * 128 + p]
                if bb >= 0:
                    sel[bb, N + j * 128 + p] = 1.0
                    sel[bb + 8, N + j * 128 + p] = 1.0
        in_maps.append({
            "na": na, "b2": b2, "naov": naov,
            "v": v[sl].reshape(128, 64),
            "y": y[sl].reshape(128, 64),
            "w": w[sl].reshape(128, 64),
            "sel": sel.astype(ml_dtypes.bfloat16), "id8": id8,
        })
    return in_maps, a_pads, overflows, ovf_batches, ovf_as


def kernel(logits, labels, weights):
    from concourse.bass_utils import run_bass_kernel_spmd

    nc = _get_nc()
    v = np.ascontiguousarray(logits.reshape(B, N), dtype=np.float32)
    y = np.ascontiguousarray(labels.reshape(B, N), dtype=np.float32)
    w = np.ascontiguousarray(weights.reshape(B, N), dtype=np.float32)

    in_maps, a_pads, overflows, ovf_batches, ovf_as = make_in_maps(v, y, w)
    res = run_bass_kernel_spmd(nc, in_maps, list(range(N_CORES)))

    # ---------- host combine ----------
    NB = NCH * BLOC
    dve_col = np.array([_engine_for(k) == "dve" for k in range(NB)])
    bce_sum = 0.0
    pair_sums = np.zeros(B, dtype=np.float64)
    for c in range(N_CORES):
        per_p = (np.asarray(res.results[c]["outd"]).astype(np.float64)
                 + np.asarray(res.results[c]["outa"]).astype(np.float64))
        r = per_p.sum(axis=0)
        bce_sum += float(r[NACC])
        cols = r[:NB].reshape(BLOC, NCH)
        chunk_a = a_pads[c].reshape(BLOC, NCH, 128).sum(axis=2)
        corr = np.where(dve_col.reshape(BLOC, NCH), KB * chunk_a, 0.0)
        pair_sums[c * BLOC:(c + 1) * BLOC] = (cols - corr).sum(axis=1)
        for j in range(NOVF):
            col = NB + j
            vals = per_p[:, col].copy()
            if _engine_for(col) == "dve":
                vals -= KB * ovf_as[c][j * 128:(j + 1) * 128]
            for p in range(128):
                bb = ovf_batches[c][j * 128 + p]
                if bb >= 0:
                    pair_sums[c * BLOC + bb] += vals[p]
        for rloc in overflows[c]:
            b = c * BLOC + rloc
            pos = v[b][y[b] == 1.0].astype(np.float64)
            neg = v[b][y[b] == 0.0].astype(np.float64) + MARGIN
            d = neg[None, :] - pos[:, None]
            pair_sums[b] = np.maximum(d, 0.0).sum()

    n_pos = y.sum(axis=1).astype(np.float64)
    n_neg = N - n_pos
    n_pairs = n_pos * n_neg
    valid = n_pairs > 0
    per_batch_mean = np.where(valid, pair_sums / np.maximum(n_pairs, 1.0), 0.0)
    valid_count = valid.sum()
    rank_loss = per_batch_mean.sum() / valid_count if valid_count > 0 else 0.0
    bce_loss = bce_sum / (B * N)
    return np.float32(bce_loss + rank_loss)



# revision 3
# speedup vs baseline: 1.5013x; 1.5013x over previous
"""Trainium2 Bass kernel for CombinedRankingLoss (BCE + pairwise margin ranking).

Full inputs: logits/labels/weights [64, 1024, 1] f32. Output: scalar f32.

Data-parallel over batch: 8 cores x 8 batches. Pairwise term per batch
    T_b = sum_{i in pos} sum_{j in neg} relu((v_j + M) - v_i)
computed via a SORTED-BAND decomposition (the loss is invariant to per-batch
candidate order, and sorting is host-side layout prep):
  - host sorts pos ascending (a) and neg+M ascending (b) per batch;
  - pos ranks go to partitions in CH=5 chunks of 128 (rank 128c+p -> partition
    p, segment c of the tile);
  - for chunk c only the neg ranks in [W0_c, W0_c+need_c) can pair
    NON-trivially with the chunk (W0_c/hi_c from searchsorted): below-window
    j have b_j <= min_a (relu = 0), above-window j have b_j >= max_a (relu
    linear -> closed form);
  - ONE PE matmul per region builds psum[p, S_c+f] = s*(b[W0_c+f] - a[128c+p])
    directly: b rows (bf16 hi+lo, selector 1) add the b values, per-chunk
    "a rows" (selector -s*a hi+lo, b2 carrying the segment indicator)
    subtract a. s = 1/n_pairs(batch) folded into all values so batches can
    share accumulator columns;
  - the linear above-window remainder sum_p [SufB_c - a_p*C_c] rides in 2
    extra columns per chunk (C split 256q + r so the bf16 coefficients are
    exact); each term is provably >= 0 for real rows and < 0 for +BIG pads,
    so the same relu reduction handles it;
  - per batch ONE relu+row-sum instruction (ACT activation Relu accum_out or
    DVE tensor_scalar max0/add accum_out) over [128, 592] consumes the tile.
    Window splits are adaptive per batch (encoded in tile CONTENT, built on
    host); only the 592-col budget is fixed. P(batch exceeds budget) < 1e-4;
    such batches fall back to exact host compute.
  - batches are paired into [128, 1184] psum tiles (3 banks, 4 bank-aligned
    matmuls: 512|80|432|160) so PE runs long uninterrupted bursts.
  - BCE via ACT Softplus (same table set as Relu -> one ACT_TABLE_LOAD) +
    3 DVE ops; weights pre-scaled by 1/(B*N) on host.
Host: sorting/searchsorted/hi-lo encoding (layout prep), fallbacks, final
scalar in f64. Device does all O(N*band) reduction work + BCE.
"""
import sys
import numpy as np

sys.path.insert(0, "/opt/trn_rl_repo")

B, N = 64, 1024
N_CORES = 8
BLOC = B // N_CORES          # batches per core
CH = 5                       # pos-rank chunks of 128 (Pa <= 640 w.p. ~1-1e-15)
NXC = 2 * CH                 # X (remainder) columns per batch
CB = 592                     # per-batch tile budget (cols): NXC + windows
PAIRW = 2 * CB               # pair tile width (3 psum banks)
ROWS = 12                    # b hi/lo + 5x a hi/lo
SELW = BLOC * 128            # 1024 selector cols
B2W = BLOC * CB              # 4736 value cols
BFW = SELW + B2W             # combined bf16 tile width
MARGIN = 0.5
BIG = 16.0                   # pad sentinel; |b| <= ~7 << BIG

_CACHE = {}


def _patch_bass(bass):
    """Split multi-wait instructions (old walrus TPB_CTRL takes 1 wait)."""
    import json as _json
    if getattr(bass.Bass, "_wait_split_patched", False):
        return
    _orig = bass.Bass.to_json_bytes

    def _split(bir, limit=1):
        m = _json.loads(bir)
        for fn in m["functions"]:
            for bb in fn["blocks"]:
                out = []
                for i in bb.get("instructions", []):
                    si = i.get("sync_info") or {}
                    ow = si.get("on_wait") or []
                    if len(ow) > limit:
                        extra, keep = ow[:-limit], ow[-limit:]
                        for k, w in enumerate(extra):
                            out.append({
                                "debug": i.get("debug"), "engine": i["engine"],
                                "ins": [], "outs": [],
                                "name": i["name"] + f"_ws{k}",
                                "opcode": "NoOp",
                                "sync_info": {"on_wait": [w]},
                            })
                        si = dict(si)
                        si["on_wait"] = keep
                        i = dict(i)
                        i["sync_info"] = si
                    out.append(i)
                bb["instructions"] = out
        return _json.dumps(m).encode()

    bass.Bass.to_json_bytes = lambda self: _split(_orig(self))
    bass.Bass._wait_split_patched = True


def _build(bass, tile, mybir):
    f32 = mybir.dt.float32
    bf16 = mybir.dt.bfloat16
    Alu = mybir.AluOpType
    Act = mybir.ActivationFunctionType

    nc = bass.Bass()
    bf_d = nc.declare_dram_parameter("bf", [ROWS, BFW], bf16, isOutput=False)
    fv_d = nc.declare_dram_parameter("fv", [128, 192], f32, isOutput=False)
    outd_d = nc.declare_dram_parameter("outd", [128, 8], f32, isOutput=True)
    outa_d = nc.declare_dram_parameter("outa", [128, 8], f32, isOutput=True)

    with tile.TileContext(nc) as tc:
        with (
            tc.tile_pool(name="const", bufs=1) as const,
            tc.tile_pool(name="work", bufs=2) as work,
            tc.tile_pool(name="psum", bufs=2, space="PSUM") as psum,
        ):
            bf = const.tile([ROWS, BFW], bf16)
            nc.sync.dma_start(out=bf[:], in_=bf_d[:])
            fv = const.tile([128, 192], f32)
            nc.sync.dma_start(out=fv[:], in_=fv_d[:])

            acc_d = const.tile([128, 8], f32)
            acc_a = const.tile([128, 8], f32)
            nc.vector.memset(acc_d[:], 0.0)
            nc.vector.memset(acc_a[:], 0.0)

            def emit_mm(t):
                """Pair tile for batches 2t, 2t+1; 4 bank-aligned matmuls."""
                bcp = psum.tile([128, PAIRW], f32, tag="bcp")
                e, o = 2 * t, 2 * t + 1
                le = bf[:, 128 * e:128 * e + 128]
                lo_ = bf[:, 128 * o:128 * o + 128]
                obe = SELW + CB * e
                obo = SELW + CB * o
                nc.tensor.matmul(bcp[:, 0:512], le, bf[:, obe:obe + 512],
                                 start=True, stop=True)
                nc.tensor.matmul(bcp[:, 512:592], le, bf[:, obe + 512:obe + 592],
                                 start=True, stop=True)
                nc.tensor.matmul(bcp[:, 592:1024], lo_, bf[:, obo:obo + 432],
                                 start=True, stop=True)
                nc.tensor.matmul(bcp[:, 1024:1184], lo_, bf[:, obo + 432:obo + 592],
                                 start=True, stop=True)
                return bcp

            def ew_dve(bcp, col):
                scr = work.tile([128, CB], f32, tag="scr_d")
                nc.vector.tensor_scalar(
                    out=scr[:], in0=bcp[:, 0:CB], scalar1=0.0, scalar2=None,
                    op0=Alu.max, op1=Alu.add, accum_out=acc_d[:, col:col + 1])

            def ew_act(bcp, col):
                scr = work.tile([128, CB], f32, tag="scr_a")
                nc.scalar.activation(
                    out=scr[:], in_=bcp[:, CB:PAIRW], func=Act.Relu,
                    accum_out=acc_a[:, col:col + 1])

            bcp0 = emit_mm(0)
            bcp1 = emit_mm(1)

            # BCE: w' * (softplus(v) - v*y), w' pre-scaled by 1/(B*N);
            # softplus = Ln(Exp(v) + 1) via Ln's bias (same ACT table as Relu)
            ex = work.tile([128, 64], f32)
            nc.scalar.activation(out=ex[:], in_=fv[:, 0:64], func=Act.Exp)
            sp = work.tile([128, 64], f32)
            nc.scalar.activation(out=sp[:], in_=ex[:], func=Act.Ln, bias=1.0)

            ew_dve(bcp0, 0)
            ew_act(bcp0, 0)

            xy = work.tile([128, 64], f32)
            nc.vector.tensor_tensor(out=xy[:], in0=fv[:, 0:64],
                                    in1=fv[:, 64:128], op=Alu.mult)
            d = work.tile([128, 64], f32)
            nc.vector.scalar_tensor_tensor(
                out=d[:], in0=xy[:], scalar=-1.0, op0=Alu.mult,
                op1=Alu.add, in1=sp[:])

            bcp2 = emit_mm(2)
            ew_dve(bcp1, 1)
            ew_act(bcp1, 1)

            bscr = work.tile([128, 64], f32)
            nc.vector.scalar_tensor_tensor(
                out=bscr[:], in0=d[:], scalar=1.0, op0=Alu.mult,
                op1=Alu.mult, in1=fv[:, 128:192],
                accum_out=acc_d[:, 4:5])

            bcp3 = emit_mm(3)
            ew_dve(bcp2, 2)
            ew_act(bcp2, 2)
            ew_dve(bcp3, 3)
            ew_act(bcp3, 3)

            nc.sync.dma_start(out=outd_d[:], in_=acc_d[:])
            nc.sync.dma_start(out=outa_d[:], in_=acc_a[:])

    return nc


def _get_nc():
    if "nc" not in _CACHE:
        import concourse.bass as bass
        import concourse.tile as tile
        from concourse import mybir
        _patch_bass(bass)
        _CACHE["nc"] = _build(bass, tile, mybir)
    return _CACHE["nc"]


def _hi_lo(x):
    """f64 array -> (bf16 hi, bf16 lo) with hi+lo ~ x to ~2^-17 rel."""
    import ml_dtypes
    hi = x.astype(np.float32).astype(ml_dtypes.bfloat16)
    lo = (x - hi.astype(np.float64)).astype(np.float32).astype(ml_dtypes.bfloat16)
    return hi, lo


def _exact_mean(pos, neg):
    """Exact per-batch pairwise mean (f64); pos/neg sorted, neg has +M."""
    if len(pos) == 0 or len(neg) == 0:
        return 0.0
    dsum = 0.0
    # chunked to keep memory small
    for i0 in range(0, len(pos), 128):
        d = neg[None, :] - pos[i0:i0 + 128, None]
        dsum += float(np.maximum(d, 0.0).sum())
    return dsum / (len(pos) * len(neg))


def _prep_batch(vrow, yrow, selblk, b2blk):
    """Fill one batch's selector [ROWS,128] and value [ROWS,CB] blocks
    (f64, hi/lo split done by caller is NOT used -- we fill final f32 content
    here and caller casts). Returns (valid, fallback_mean_or_None)."""
    pos = np.sort(vrow[yrow == 1.0]).astype(np.float64)
    neg = np.sort(vrow[yrow == 0.0]).astype(np.float64) + MARGIN
    Pa, Nb = len(pos), len(neg)
    n_pairs = Pa * Nb
    if n_pairs == 0:
        return False, None            # invalid batch: zero content, mean 0
    if Pa > CH * 128:
        return True, _exact_mean(pos, neg)
    s = 1.0 / n_pairs

    W0s, needs = [], []
    for c in range(CH):
        lo_r = 128 * c
        if lo_r >= Pa:
            W0s.append(Nb)
            needs.append(0)
            continue
        hi_r = min(lo_r + 127, Pa - 1)
        w0 = int(np.searchsorted(neg, pos[lo_r], 'left'))
        hi = int(np.searchsorted(neg, pos[hi_r], 'right'))
        W0s.append(w0)
        needs.append(hi - w0)
    if sum(needs) + NXC > CB:
        return True, _exact_mean(pos, neg)

    negs = neg * s
    bval = np.zeros(CB, dtype=np.float64)     # b-row content (pre hi/lo)
    aind = np.zeros((CH, CB), dtype=np.float64)  # a-row indicator/coef rows
    avals = np.full((CH, 128), -s * BIG, dtype=np.float64)  # -s*a per chunk

    col = NXC
    for c in range(CH):
        w0, nd = W0s[c], needs[c]
        lo_r = 128 * c
        cnt = max(0, min(128, Pa - lo_r))
        if cnt > 0:
            avals[c, :cnt] = -s * pos[lo_r:lo_r + cnt]
        if nd > 0:
            bval[col:col + nd] = negs[w0:w0 + nd]
            aind[c, col:col + nd] = 1.0
        # remainder: C fully-active neg above the window
        E = w0 + nd
        C = Nb - E
        if C > 0:
            sufb = float(negs[E:].sum())
            q, r = C >> 8, C & 255
            xq, xr = 2 * c, 2 * c + 1
            if q > 0:
                bval[xq] = sufb * (256.0 * q / C)
                aind[c, xq] = 256.0 * q
            if r > 0:
                bval[xr] = sufb * (r / C)
                aind[c, xr] = r
        col += nd

    bhi, blo = _hi_lo(bval)
    b2blk[0, :] = bhi
    b2blk[1, :] = blo
    ahi, alo = _hi_lo(avals)
    for c in range(CH):
        b2blk[2 + 2 * c, :] = aind[c].astype(np.float32)
        b2blk[3 + 2 * c, :] = aind[c].astype(np.float32)
        selblk[2 + 2 * c, :] = ahi[c]
        selblk[3 + 2 * c, :] = alo[c]
    selblk[0, :] = 1.0
    selblk[1, :] = 1.0
    return True, None


def make_in_maps(v, y, w):
    import ml_dtypes
    in_maps, aux = [], []
    wsc = (w.astype(np.float64) / (B * N)).astype(np.float32)
    for core in range(N_CORES):
        sl = slice(core * BLOC, (core + 1) * BLOC)
        vb, yb, wb = v[sl], y[sl], wsc[sl]
        bft = np.zeros((ROWS, BFW), dtype=ml_dtypes.bfloat16)
        extra_mean = 0.0
        n_valid = 0
        for b in range(BLOC):
            selblk = np.zeros((ROWS, 128), dtype=ml_dtypes.bfloat16)
            b2blk = np.zeros((ROWS, CB), dtype=ml_dtypes.bfloat16)
            valid, fb = _prep_batch(vb[b], yb[b], selblk, b2blk)
            if valid:
                n_valid += 1
            if fb is not None:
                extra_mean += fb      # fallback: host-exact, zero content
            else:
                bft[:, 128 * b:128 * b + 128] = selblk
                bft[:, SELW + CB * b:SELW + CB * (b + 1)] = b2blk
        fvt = np.concatenate(
            [vb.reshape(128, 64), yb.reshape(128, 64), wb.reshape(128, 64)],
            axis=1).astype(np.float32)
        in_maps.append({"bf": bft, "fv": np.ascontiguousarray(fvt)})
        aux.append({"extra_mean": extra_mean, "n_valid": n_valid})
    return in_maps, aux


def kernel(logits, labels, weights):
    from concourse.bass_utils import run_bass_kernel_spmd

    nc = _get_nc()
    v = np.ascontiguousarray(logits.reshape(B, N), dtype=np.float32)
    y = np.ascontiguousarray(labels.reshape(B, N), dtype=np.float32)
    w = np.ascontiguousarray(weights.reshape(B, N), dtype=np.float32)

    in_maps, aux = make_in_maps(v, y, w)
    res = run_bass_kernel_spmd(nc, in_maps, list(range(N_CORES)))

    mean_sum = 0.0
    bce_sum = 0.0
    valid_count = 0
    for c in range(N_CORES):
        od = np.asarray(res.results[c]["outd"]).astype(np.float64)
        oa = np.asarray(res.results[c]["outa"]).astype(np.float64)
        mean_sum += od[:, 0:4].sum() + oa[:, 0:4].sum()
        bce_sum += od[:, 4].sum()
        mean_sum += aux[c]["extra_mean"]
        valid_count += aux[c]["n_valid"]
    rank_loss = mean_sum / valid_count if valid_count > 0 else 0.0
    return np.float32(bce_sum + rank_loss)


# revision 7
# speedup vs baseline: 1.6218x; 1.0802x over previous
"""Trainium2 Bass kernel for CombinedRankingLoss (BCE + pairwise margin ranking).

Full inputs: logits/labels/weights [64, 1024, 1] f32. Output: scalar f32.

Data-parallel over batch: 8 cores x 8 batches. Pairwise term per batch
    T_b = sum_{i in pos} sum_{j in neg} relu((v_j + M) - v_i)
computed via a SORTED-BAND decomposition (the loss is invariant to per-batch
candidate order, and sorting is host-side layout prep):
  - host sorts pos ascending (a) and neg+M ascending (b) per batch;
  - pos ranks go to partitions in CH=5 chunks of 128 (rank 128c+p -> partition
    p, segment c of the tile);
  - for chunk c only the neg ranks in [W0_c, W0_c+need_c) can pair
    NON-trivially with the chunk (W0_c/hi_c from searchsorted): below-window
    j have b_j <= min_a (relu = 0), above-window j have b_j >= max_a (relu
    linear -> closed form);
  - ONE PE matmul per region builds psum[p, S_c+f] = s*(b[W0_c+f] - a[128c+p])
    directly: b rows (bf16 hi+lo, selector 1) add the b values, per-chunk
    "a rows" (selector -s*a hi+lo, b2 carrying the segment indicator)
    subtract a. s = 1/n_pairs(batch) folded into all values so batches can
    share accumulator columns;
  - the linear above-window remainder sum_p [SufB_c - a_p*C_c] rides in 2
    extra columns per chunk (C split 256q + r so the bf16 coefficients are
    exact); each term is provably >= 0 for real rows and < 0 for +BIG pads,
    so the same relu reduction handles it;
  - per batch ONE relu+row-sum instruction (ACT activation Relu accum_out or
    DVE tensor_scalar max0/add accum_out) over [128, 592] consumes the tile.
    Window splits are adaptive per batch (encoded in tile CONTENT, built on
    host); only the 592-col budget is fixed. P(batch exceeds budget) < 1e-4;
    such batches fall back to exact host compute.
  - batches are paired into [128, 1184] psum tiles (3 banks, 4 bank-aligned
    matmuls: 512|80|432|160) so PE runs long uninterrupted bursts.
  - BCE via ACT Softplus (same table set as Relu -> one ACT_TABLE_LOAD) +
    3 DVE ops; weights pre-scaled by 1/(B*N) on host.
Host: sorting/searchsorted/hi-lo encoding (layout prep), fallbacks, final
scalar in f64. Device does all O(N*band) reduction work + BCE.
"""
import sys
import numpy as np

sys.path.insert(0, "/opt/trn_rl_repo")

B, N = 64, 1024
N_CORES = 8
BLOC = B // N_CORES          # batches per core
CH = 5                       # pos-rank chunks of 128 (Pa <= 640 w.p. ~1-1e-15)
NXC = 2 * CH                 # X (remainder) columns per batch
CB = 592                     # per-batch tile budget (cols): NXC + windows
PAIRW = 2 * CB               # pair tile width (3 psum banks)
ROWS = 12                    # b hi/lo + 5x a hi/lo
SELW = BLOC * 128            # 1024 selector cols
B2W = BLOC * CB              # 4736 value cols
BFW = SELW + B2W             # combined bf16 tile width
MARGIN = 0.5
BIG = 16.0                   # pad sentinel; |b| <= ~7 << BIG

_CACHE = {}


def _patch_bass(bass):
    """Split multi-wait instructions (old walrus TPB_CTRL takes 1 wait)."""
    import json as _json
    if getattr(bass.Bass, "_wait_split_patched", False):
        return
    _orig = bass.Bass.to_json_bytes

    def _split(bir, limit=1):
        m = _json.loads(bir)
        for fn in m["functions"]:
            for bb in fn["blocks"]:
                out = []
                for i in bb.get("instructions", []):
                    si = i.get("sync_info") or {}
                    ow = si.get("on_wait") or []
                    if len(ow) > limit:
                        extra, keep = ow[:-limit], ow[-limit:]
                        for k, w in enumerate(extra):
                            out.append({
                                "debug": i.get("debug"), "engine": i["engine"],
                                "ins": [], "outs": [],
                                "name": i["name"] + f"_ws{k}",
                                "opcode": "NoOp",
                                "sync_info": {"on_wait": [w]},
                            })
                        si = dict(si)
                        si["on_wait"] = keep
                        i = dict(i)
                        i["sync_info"] = si
                    out.append(i)
                bb["instructions"] = out
        return _json.dumps(m).encode()

    bass.Bass.to_json_bytes = lambda self: _split(_orig(self))
    bass.Bass._wait_split_patched = True


def _build(bass, tile, mybir):
    f32 = mybir.dt.float32
    bf16 = mybir.dt.bfloat16
    Alu = mybir.AluOpType
    Act = mybir.ActivationFunctionType

    nc = bass.Bass()
    bf_d = nc.declare_dram_parameter("bf", [ROWS, BFW], bf16, isOutput=False)
    fv_d = nc.declare_dram_parameter("fv", [128, 192], f32, isOutput=False)
    outd_d = nc.declare_dram_parameter("outd", [128, 8], f32, isOutput=True)
    outa_d = nc.declare_dram_parameter("outa", [128, 8], f32, isOutput=True)

    with tile.TileContext(nc) as tc:
        with (
            tc.tile_pool(name="const", bufs=1) as const,
            tc.tile_pool(name="work", bufs=2) as work,
            tc.tile_pool(name="psum", bufs=2, space="PSUM") as psum,
        ):
            bf = const.tile([ROWS, BFW], bf16)
            nc.sync.dma_start(out=bf[:], in_=bf_d[:])
            fv = const.tile([128, 192], f32)
            nc.sync.dma_start(out=fv[:], in_=fv_d[:])

            acc_d = const.tile([128, 8], f32)
            acc_a = const.tile([128, 8], f32)
            nc.vector.memset(acc_d[:], 0.0)
            nc.vector.memset(acc_a[:], 0.0)

            # pair cols [0:1184) split 704 (DVE) / 480 (ACT) across two psum
            # tiles so the two consumers never read the SAME tile (the tile
            # framework serializes same-tile readers across engines).
            DW = 704

            def emit_mm(t):
                """Pair tiles for batches 2t, 2t+1; 4 bank-contained matmuls."""
                bcd = psum.tile([128, DW], f32, tag="bcd")
                bca = psum.tile([128, PAIRW - DW], f32, tag="bca")
                e, o = 2 * t, 2 * t + 1
                le = bf[:, 128 * e:128 * e + 128]
                lo_ = bf[:, 128 * o:128 * o + 128]
                obe = SELW + CB * e
                obo = SELW + CB * o
                nc.tensor.matmul(bcd[:, 0:512], le, bf[:, obe:obe + 512],
                                 start=True, stop=True)
                nc.tensor.matmul(bcd[:, 512:592], le, bf[:, obe + 512:obe + 592],
                                 start=True, stop=True)
                nc.tensor.matmul(bcd[:, 592:DW], lo_, bf[:, obo:obo + (DW - 592)],
                                 start=True, stop=True)
                nc.tensor.matmul(bca[:, 0:PAIRW - DW], lo_,
                                 bf[:, obo + (DW - 592):obo + CB],
                                 start=True, stop=True)
                return bcd, bca

            def ew_dve(bcd, col):
                scr = work.tile([128, DW], f32, tag="scr_d")
                nc.vector.tensor_scalar(
                    out=scr[:], in0=bcd[:], scalar1=0.0, scalar2=None,
                    op0=Alu.max, op1=Alu.add, accum_out=acc_d[:, col:col + 1])

            def ew_act(bca, col):
                scr = work.tile([128, PAIRW - DW], f32, tag="scr_a")
                nc.scalar.activation(
                    out=scr[:], in_=bca[:], func=Act.Relu,
                    accum_out=acc_a[:, col:col + 1])

            p0 = emit_mm(0)
            p1 = emit_mm(1)

            # BCE: sum w'*(softplus(v) - v*y) = sum w'*sp - sum wy*v with
            # wy = w'*y host-precomputed; softplus = Ln(Exp(v) + 1) via Ln's
            # bias (natural_log_exp table also holds Relu -> one table load)
            ex = work.tile([128, 64], f32)
            nc.scalar.activation(out=ex[:], in_=fv[:, 0:64], func=Act.Exp)
            sp = work.tile([128, 64], f32)
            nc.scalar.activation(out=sp[:], in_=ex[:], func=Act.Ln, bias=1.0)

            ew_dve(p0[0], 0)
            ew_act(p0[1], 0)

            b1 = work.tile([128, 64], f32)
            nc.vector.scalar_tensor_tensor(
                out=b1[:], in0=sp[:], scalar=1.0, op0=Alu.mult,
                op1=Alu.mult, in1=fv[:, 128:192], accum_out=acc_d[:, 4:5])
            b2s = work.tile([128, 64], f32)
            nc.vector.scalar_tensor_tensor(
                out=b2s[:], in0=fv[:, 0:64], scalar=-1.0, op0=Alu.mult,
                op1=Alu.mult, in1=fv[:, 64:128], accum_out=acc_d[:, 5:6])

            p2 = emit_mm(2)
            ew_dve(p1[0], 1)
            ew_act(p1[1], 1)
            p3 = emit_mm(3)
            ew_dve(p2[0], 2)
            ew_act(p2[1], 2)
            ew_dve(p3[0], 3)
            ew_act(p3[1], 3)

            nc.sync.dma_start(out=outd_d[:], in_=acc_d[:])
            nc.sync.dma_start(out=outa_d[:], in_=acc_a[:])

    return nc


def _get_nc():
    if "nc" not in _CACHE:
        import concourse.bass as bass
        import concourse.tile as tile
        from concourse import mybir
        _patch_bass(bass)
        _CACHE["nc"] = _build(bass, tile, mybir)
    return _CACHE["nc"]


def _hi_lo(x):
    """f64 array -> (bf16 hi, bf16 lo) with hi+lo ~ x to ~2^-17 rel."""
    import ml_dtypes
    hi = x.astype(np.float32).astype(ml_dtypes.bfloat16)
    lo = (x - hi.astype(np.float64)).astype(np.float32).astype(ml_dtypes.bfloat16)
    return hi, lo


def _exact_mean(pos, neg):
    """Exact per-batch pairwise mean (f64); pos/neg sorted, neg has +M."""
    if len(pos) == 0 or len(neg) == 0:
        return 0.0
    dsum = 0.0
    # chunked to keep memory small
    for i0 in range(0, len(pos), 128):
        d = neg[None, :] - pos[i0:i0 + 128, None]
        dsum += float(np.maximum(d, 0.0).sum())
    return dsum / (len(pos) * len(neg))


def _prep_batch(vrow, yrow, selblk, b2blk):
    """Fill one batch's selector [ROWS,128] and value [ROWS,CB] blocks
    (f64, hi/lo split done by caller is NOT used -- we fill final f32 content
    here and caller casts). Returns (valid, fallback_mean_or_None)."""
    pos = np.sort(vrow[yrow == 1.0]).astype(np.float64)
    neg = np.sort(vrow[yrow == 0.0]).astype(np.float64) + MARGIN
    Pa, Nb = len(pos), len(neg)
    n_pairs = Pa * Nb
    if n_pairs == 0:
        return False, None            # invalid batch: zero content, mean 0
    if Pa > CH * 128:
        return True, _exact_mean(pos, neg)
    s = 1.0 / n_pairs

    W0s, needs = [], []
    for c in range(CH):
        lo_r = 128 * c
        if lo_r >= Pa:
            W0s.append(Nb)
            needs.append(0)
            continue
        hi_r = min(lo_r + 127, Pa - 1)
        w0 = int(np.searchsorted(neg, pos[lo_r], 'left'))
        hi = int(np.searchsorted(neg, pos[hi_r], 'right'))
        W0s.append(w0)
        needs.append(hi - w0)
    if sum(needs) + NXC > CB:
        return True, _exact_mean(pos, neg)

    negs = neg * s
    bval = np.zeros(CB, dtype=np.float64)     # b-row content (pre hi/lo)
    aind = np.zeros((CH, CB), dtype=np.float64)  # a-row indicator/coef rows
    avals = np.full((CH, 128), -s * BIG, dtype=np.float64)  # -s*a per chunk

    col = NXC
    for c in range(CH):
        w0, nd = W0s[c], needs[c]
        lo_r = 128 * c
        cnt = max(0, min(128, Pa - lo_r))
        if cnt > 0:
            avals[c, :cnt] = -s * pos[lo_r:lo_r + cnt]
        if nd > 0:
            bval[col:col + nd] = negs[w0:w0 + nd]
            aind[c, col:col + nd] = 1.0
        # remainder: C fully-active neg above the window
        E = w0 + nd
        C = Nb - E
        if C > 0:
            sufb = float(negs[E:].sum())
            q, r = C >> 8, C & 255
            xq, xr = 2 * c, 2 * c + 1
            if q > 0:
                bval[xq] = sufb * (256.0 * q / C)
                aind[c, xq] = 256.0 * q
            if r > 0:
                bval[xr] = sufb * (r / C)
                aind[c, xr] = r
        col += nd

    bhi, blo = _hi_lo(bval)
    b2blk[0, :] = bhi
    b2blk[1, :] = blo
    ahi, alo = _hi_lo(avals)
    for c in range(CH):
        b2blk[2 + 2 * c, :] = aind[c].astype(np.float32)
        b2blk[3 + 2 * c, :] = aind[c].astype(np.float32)
        selblk[2 + 2 * c, :] = ahi[c]
        selblk[3 + 2 * c, :] = alo[c]
    selblk[0, :] = 1.0
    selblk[1, :] = 1.0
    return True, None


def make_in_maps(v, y, w):
    import ml_dtypes
    in_maps, aux = [], []
    wsc = (w.astype(np.float64) / (B * N)).astype(np.float32)
    for core in range(N_CORES):
        sl = slice(core * BLOC, (core + 1) * BLOC)
        vb, yb, wb = v[sl], y[sl], wsc[sl]
        bft = np.zeros((ROWS, BFW), dtype=ml_dtypes.bfloat16)
        extra_mean = 0.0
        n_valid = 0
        for b in range(BLOC):
            selblk = np.zeros((ROWS, 128), dtype=ml_dtypes.bfloat16)
            b2blk = np.zeros((ROWS, CB), dtype=ml_dtypes.bfloat16)
            valid, fb = _prep_batch(vb[b], yb[b], selblk, b2blk)
            if valid:
                n_valid += 1
            if fb is not None:
                extra_mean += fb      # fallback: host-exact, zero content
            else:
                bft[:, 128 * b:128 * b + 128] = selblk
                bft[:, SELW + CB * b:SELW + CB * (b + 1)] = b2blk
        wy = (wb.astype(np.float64) * yb).astype(np.float32)
        fvt = np.concatenate(
            [vb.reshape(128, 64), wy.reshape(128, 64), wb.reshape(128, 64)],
            axis=1).astype(np.float32)
        in_maps.append({"bf": bft, "fv": np.ascontiguousarray(fvt)})
        aux.append({"extra_mean": extra_mean, "n_valid": n_valid})
    return in_maps, aux


def kernel(logits, labels, weights):
    from concourse.bass_utils import run_bass_kernel_spmd

    nc = _get_nc()
    v = np.ascontiguousarray(logits.reshape(B, N), dtype=np.float32)
    y = np.ascontiguousarray(labels.reshape(B, N), dtype=np.float32)
    w = np.ascontiguousarray(weights.reshape(B, N), dtype=np.float32)

    in_maps, aux = make_in_maps(v, y, w)
    res = run_bass_kernel_spmd(nc, in_maps, list(range(N_CORES)))

    mean_sum = 0.0
    bce_sum = 0.0
    valid_count = 0
    for c in range(N_CORES):
        od = np.asarray(res.results[c]["outd"]).astype(np.float64)
        oa = np.asarray(res.results[c]["outa"]).astype(np.float64)
        mean_sum += od[:, 0:4].sum() + oa[:, 0:4].sum()
        bce_sum += od[:, 4].sum() + od[:, 5].sum()
        mean_sum += aux[c]["extra_mean"]
        valid_count += aux[c]["n_valid"]
    rank_loss = mean_sum / valid_count if valid_count > 0 else 0.0
    return np.float32(bce_sum + rank_loss)


# revision 11
# speedup vs baseline: 1.6344x; 1.0078x over previous
"""Trainium2 Bass kernel for CombinedRankingLoss (BCE + pairwise margin ranking).

Full inputs: logits/labels/weights [64, 1024, 1] f32. Output: scalar f32.

Data-parallel over batch: 8 cores x 8 batches. Pairwise term per batch
    T_b = sum_{i in pos} sum_{j in neg} relu((v_j + M) - v_i)
computed via a SORTED-BAND decomposition (the loss is invariant to per-batch
candidate order, and sorting is host-side layout prep):
  - host sorts pos ascending (a) and neg+M ascending (b) per batch;
  - pos ranks go to partitions in CH=5 chunks of 128 (rank 128c+p -> partition
    p, segment c of the tile);
  - for chunk c only the neg ranks in [W0_c, W0_c+need_c) can pair
    NON-trivially with the chunk (W0_c/hi_c from searchsorted): below-window
    j have b_j <= min_a (relu = 0), above-window j have b_j >= max_a (relu
    linear -> closed form);
  - ONE PE matmul per region builds psum[p, S_c+f] = s*(b[W0_c+f] - a[128c+p])
    directly: b rows (bf16 hi+lo, selector 1) add the b values, per-chunk
    "a rows" (selector -s*a hi+lo, b2 carrying the segment indicator)
    subtract a. s = 1/n_pairs(batch) folded into all values so batches can
    share accumulator columns;
  - the linear above-window remainder sum_p [SufB_c - a_p*C_c] rides in 2
    extra columns per chunk (C split 256q + r so the bf16 coefficients are
    exact); each term is provably >= 0 for real rows and < 0 for +BIG pads,
    so the same relu reduction handles it;
  - per batch ONE relu+row-sum instruction (ACT activation Relu accum_out or
    DVE tensor_scalar max0/add accum_out) over [128, 592] consumes the tile.
    Window splits are adaptive per batch (encoded in tile CONTENT, built on
    host); only the 592-col budget is fixed. P(batch exceeds budget) < 1e-4;
    such batches fall back to exact host compute.
  - batches are paired into [128, 1184] psum tiles (3 banks, 4 bank-aligned
    matmuls: 512|80|432|160) so PE runs long uninterrupted bursts.
  - BCE via ACT Softplus (same table set as Relu -> one ACT_TABLE_LOAD) +
    3 DVE ops; weights pre-scaled by 1/(B*N) on host.
Host: sorting/searchsorted/hi-lo encoding (layout prep), fallbacks, final
scalar in f64. Device does all O(N*band) reduction work + BCE.
"""
import sys
import numpy as np

sys.path.insert(0, "/opt/trn_rl_repo")

B, N = 64, 1024
N_CORES = 8
BLOC = B // N_CORES          # batches per core
CH = 5                       # pos-rank chunks of 128 (Pa <= 640 w.p. ~1-1e-15)
NXC = 2 * CH                 # X (remainder) columns per batch
CB = 592                     # per-batch tile budget (cols): NXC + windows
PAIRW = 2 * CB               # pair tile width (3 psum banks)
ROWS = 12                    # b hi/lo + 5x a hi/lo
SELW = BLOC * 128            # 1024 selector cols
B2W = BLOC * CB              # 4736 value cols
BFW = SELW + B2W             # combined bf16 tile width
MARGIN = 0.5
BIG = 16.0                   # pad sentinel; |b| <= ~7 << BIG

_CACHE = {}


def _patch_bass(bass):
    """Split multi-wait instructions (old walrus TPB_CTRL takes 1 wait)."""
    import json as _json
    if getattr(bass.Bass, "_wait_split_patched", False):
        return
    _orig = bass.Bass.to_json_bytes

    def _split(bir, limit=1):
        m = _json.loads(bir)
        for fn in m["functions"]:
            for bb in fn["blocks"]:
                out = []
                for i in bb.get("instructions", []):
                    si = i.get("sync_info") or {}
                    ow = si.get("on_wait") or []
                    if len(ow) > limit:
                        extra, keep = ow[:-limit], ow[-limit:]
                        for k, w in enumerate(extra):
                            out.append({
                                "debug": i.get("debug"), "engine": i["engine"],
                                "ins": [], "outs": [],
                                "name": i["name"] + f"_ws{k}",
                                "opcode": "NoOp",
                                "sync_info": {"on_wait": [w]},
                            })
                        si = dict(si)
                        si["on_wait"] = keep
                        i = dict(i)
                        i["sync_info"] = si
                    out.append(i)
                bb["instructions"] = out
        return _json.dumps(m).encode()

    bass.Bass.to_json_bytes = lambda self: _split(_orig(self))
    bass.Bass._wait_split_patched = True


def _build(bass, tile, mybir):
    f32 = mybir.dt.float32
    bf16 = mybir.dt.bfloat16
    Alu = mybir.AluOpType
    Act = mybir.ActivationFunctionType

    nc = bass.Bass()
    bf_d = nc.declare_dram_parameter("bf", [ROWS, BFW], bf16, isOutput=False)
    fv_d = nc.declare_dram_parameter("fv", [128, 192], f32, isOutput=False)
    outd_d = nc.declare_dram_parameter("outd", [128, 8], f32, isOutput=True)
    outa_d = nc.declare_dram_parameter("outa", [128, 8], f32, isOutput=True)

    with tile.TileContext(nc) as tc:
        with (
            tc.tile_pool(name="const", bufs=1) as const,
            tc.tile_pool(name="work", bufs=2) as work,
            tc.tile_pool(name="psum", bufs=2, space="PSUM") as psum,
        ):
            # engine-issued DMAs: the two HWDGE configs run in parallel on
            # the DVE/ACT sequencers instead of serially on Sync
            bf = const.tile([ROWS, BFW], bf16)
            nc.sync.dma_start(out=bf[:], in_=bf_d[:])
            fv = const.tile([128, 192], f32)
            nc.scalar.dma_start(out=fv[:], in_=fv_d[:])

            acc_d = const.tile([128, 8], f32)
            acc_a = const.tile([128, 8], f32)
            nc.vector.memset(acc_d[:], 0.0)
            nc.vector.memset(acc_a[:], 0.0)

            # pair cols [0:1184) split 704 (DVE) / 480 (ACT) across two psum
            # tiles so the two consumers never read the SAME tile (the tile
            # framework serializes same-tile readers across engines).
            DW = 704

            def emit_mm(t):
                """Pair tiles for batches 2t, 2t+1; 4 bank-contained matmuls."""
                bcd = psum.tile([128, DW], f32, tag="bcd")
                bca = psum.tile([128, PAIRW - DW], f32, tag="bca")
                e, o = 2 * t, 2 * t + 1
                le = bf[:, 128 * e:128 * e + 128]
                lo_ = bf[:, 128 * o:128 * o + 128]
                obe = SELW + CB * e
                obo = SELW + CB * o
                nc.tensor.matmul(bcd[:, 0:512], le, bf[:, obe:obe + 512],
                                 start=True, stop=True)
                nc.tensor.matmul(bcd[:, 512:592], le, bf[:, obe + 512:obe + 592],
                                 start=True, stop=True)
                nc.tensor.matmul(bcd[:, 592:DW], lo_, bf[:, obo:obo + (DW - 592)],
                                 start=True, stop=True)
                nc.tensor.matmul(bca[:, 0:PAIRW - DW], lo_,
                                 bf[:, obo + (DW - 592):obo + CB],
                                 start=True, stop=True)
                return bcd, bca

            def ew_dve(bcd, col):
                scr = work.tile([128, DW], f32, tag="scr_d")
                nc.vector.tensor_scalar(
                    out=scr[:], in0=bcd[:], scalar1=0.0, scalar2=None,
                    op0=Alu.max, op1=Alu.add, accum_out=acc_d[:, col:col + 1])

            def ew_act(bca, col):
                scr = work.tile([128, PAIRW - DW], f32, tag="scr_a")
                nc.scalar.activation(
                    out=scr[:], in_=bca[:], func=Act.Relu,
                    accum_out=acc_a[:, col:col + 1])

            p0 = emit_mm(0)
            p1 = emit_mm(1)

            # BCE: sum w'*(softplus(v) - v*y) = sum w'*sp - sum wy*v with
            # wy = w'*y host-precomputed; softplus = Ln(Exp(v) + 1) via Ln's
            # bias (natural_log_exp table also holds Relu -> one table load)
            ex = work.tile([128, 64], f32)
            nc.scalar.activation(out=ex[:], in_=fv[:, 0:64], func=Act.Exp)
            sp = work.tile([128, 64], f32)
            nc.scalar.activation(out=sp[:], in_=ex[:], func=Act.Ln, bias=1.0)

            ew_dve(p0[0], 0)
            ew_act(p0[1], 0)

            b1 = work.tile([128, 64], f32)
            nc.vector.scalar_tensor_tensor(
                out=b1[:], in0=sp[:], scalar=1.0, op0=Alu.mult,
                op1=Alu.mult, in1=fv[:, 128:192], accum_out=acc_d[:, 4:5])
            b2s = work.tile([128, 64], f32)
            nc.vector.scalar_tensor_tensor(
                out=b2s[:], in0=fv[:, 0:64], scalar=-1.0, op0=Alu.mult,
                op1=Alu.mult, in1=fv[:, 64:128], accum_out=acc_d[:, 5:6])

            p2 = emit_mm(2)
            ew_dve(p1[0], 1)
            ew_act(p1[1], 1)
            p3 = emit_mm(3)
            ew_dve(p2[0], 2)
            ew_act(p2[1], 2)
            ew_dve(p3[0], 3)
            ew_act(p3[1], 3)

            nc.sync.dma_start(out=outd_d[:], in_=acc_d[:])
            nc.scalar.dma_start(out=outa_d[:], in_=acc_a[:])

    return nc


def _get_nc():
    if "nc" not in _CACHE:
        import concourse.bass as bass
        import concourse.tile as tile
        from concourse import mybir
        _patch_bass(bass)
        _CACHE["nc"] = _build(bass, tile, mybir)
    return _CACHE["nc"]


def _hi_lo(x):
    """f64 array -> (bf16 hi, bf16 lo) with hi+lo ~ x to ~2^-17 rel."""
    import ml_dtypes
    hi = x.astype(np.float32).astype(ml_dtypes.bfloat16)
    lo = (x - hi.astype(np.float64)).astype(np.float32).astype(ml_dtypes.bfloat16)
    return hi, lo


def _exact_mean(pos, neg):
    """Exact per-batch pairwise mean (f64); pos/neg sorted, neg has +M."""
    if len(pos) == 0 or len(neg) == 0:
        return 0.0
    dsum = 0.0
    # chunked to keep memory small
    for i0 in range(0, len(pos), 128):
        d = neg[None, :] - pos[i0:i0 + 128, None]
        dsum += float(np.maximum(d, 0.0).sum())
    return dsum / (len(pos) * len(neg))


def _prep_batch(vrow, yrow, selblk, b2blk):
    """Fill one batch's selector [ROWS,128] and value [ROWS,CB] blocks
    (f64, hi/lo split done by caller is NOT used -- we fill final f32 content
    here and caller casts). Returns (valid, fallback_mean_or_None)."""
    pos = np.sort(vrow[yrow == 1.0]).astype(np.float64)
    neg = np.sort(vrow[yrow == 0.0]).astype(np.float64) + MARGIN
    Pa, Nb = len(pos), len(neg)
    n_pairs = Pa * Nb
    if n_pairs == 0:
        return False, None            # invalid batch: zero content, mean 0
    if Pa > CH * 128:
        return True, _exact_mean(pos, neg)
    s = 1.0 / n_pairs

    W0s, needs = [], []
    for c in range(CH):
        lo_r = 128 * c
        if lo_r >= Pa:
            W0s.append(Nb)
            needs.append(0)
            continue
        hi_r = min(lo_r + 127, Pa - 1)
        w0 = int(np.searchsorted(neg, pos[lo_r], 'left'))
        hi = int(np.searchsorted(neg, pos[hi_r], 'right'))
        W0s.append(w0)
        needs.append(hi - w0)
    if sum(needs) + NXC > CB:
        return True, _exact_mean(pos, neg)

    negs = neg * s
    bval = np.zeros(CB, dtype=np.float64)     # b-row content (pre hi/lo)
    aind = np.zeros((CH, CB), dtype=np.float64)  # a-row indicator/coef rows
    avals = np.full((CH, 128), -s * BIG, dtype=np.float64)  # -s*a per chunk

    col = NXC
    for c in range(CH):
        w0, nd = W0s[c], needs[c]
        lo_r = 128 * c
        cnt = max(0, min(128, Pa - lo_r))
        if cnt > 0:
            avals[c, :cnt] = -s * pos[lo_r:lo_r + cnt]
        if nd > 0:
            bval[col:col + nd] = negs[w0:w0 + nd]
            aind[c, col:col + nd] = 1.0
        # remainder: C fully-active neg above the window
        E = w0 + nd
        C = Nb - E
        if C > 0:
            sufb = float(negs[E:].sum())
            q, r = C >> 8, C & 255
            xq, xr = 2 * c, 2 * c + 1
            if q > 0:
                bval[xq] = sufb * (256.0 * q / C)
                aind[c, xq] = 256.0 * q
            if r > 0:
                bval[xr] = sufb * (r / C)
                aind[c, xr] = r
        col += nd

    bhi, blo = _hi_lo(bval)
    b2blk[0, :] = bhi
    b2blk[1, :] = blo
    ahi, alo = _hi_lo(avals)
    for c in range(CH):
        b2blk[2 + 2 * c, :] = aind[c].astype(np.float32)
        b2blk[3 + 2 * c, :] = aind[c].astype(np.float32)
        selblk[2 + 2 * c, :] = ahi[c]
        selblk[3 + 2 * c, :] = alo[c]
    selblk[0, :] = 1.0
    selblk[1, :] = 1.0
    return True, None


def make_in_maps(v, y, w):
    import ml_dtypes
    in_maps, aux = [], []
    wsc = (w.astype(np.float64) / (B * N)).astype(np.float32)
    for core in range(N_CORES):
        sl = slice(core * BLOC, (core + 1) * BLOC)
        vb, yb, wb = v[sl], y[sl], wsc[sl]
        bft = np.zeros((ROWS, BFW), dtype=ml_dtypes.bfloat16)
        extra_mean = 0.0
        n_valid = 0
        for b in range(BLOC):
            selblk = np.zeros((ROWS, 128), dtype=ml_dtypes.bfloat16)
            b2blk = np.zeros((ROWS, CB), dtype=ml_dtypes.bfloat16)
            valid, fb = _prep_batch(vb[b], yb[b], selblk, b2blk)
            if valid:
                n_valid += 1
            if fb is not None:
                extra_mean += fb      # fallback: host-exact, zero content
            else:
                bft[:, 128 * b:128 * b + 128] = selblk
                bft[:, SELW + CB * b:SELW + CB * (b + 1)] = b2blk
        wy = (wb.astype(np.float64) * yb).astype(np.float32)
        fvt = np.concatenate(
            [vb.reshape(128, 64), wy.reshape(128, 64), wb.reshape(128, 64)],
            axis=1).astype(np.float32)
        in_maps.append({"bf": bft, "fv": np.ascontiguousarray(fvt)})
        aux.append({"extra_mean": extra_mean, "n_valid": n_valid})
    return in_maps, aux


def kernel(logits, labels, weights):
    from concourse.bass_utils import run_bass_kernel_spmd

    nc = _get_nc()
    v = np.ascontiguousarray(logits.reshape(B, N), dtype=np.float32)
    y = np.ascontiguousarray(labels.reshape(B, N), dtype=np.float32)
    w = np.ascontiguousarray(weights.reshape(B, N), dtype=np.float32)

    in_maps, aux = make_in_maps(v, y, w)
    res = run_bass_kernel_spmd(nc, in_maps, list(range(N_CORES)))

    mean_sum = 0.0
    bce_sum = 0.0
    valid_count = 0
    for c in range(N_CORES):
        od = np.asarray(res.results[c]["outd"]).astype(np.float64)
        oa = np.asarray(res.results[c]["outa"]).astype(np.float64)
        mean_sum += od[:, 0:4].sum() + oa[:, 0:4].sum()
        bce_sum += od[:, 4].sum() + od[:, 5].sum()
        mean_sum += aux[c]["extra_mean"]
        valid_count += aux[c]["n_valid"]
    rank_loss = mean_sum / valid_count if valid_count > 0 else 0.0
    return np.float32(bce_sum + rank_loss)


# revision 13
# speedup vs baseline: 1.6568x; 1.0137x over previous
"""Trainium2 Bass kernel for CombinedRankingLoss (BCE + pairwise margin ranking).

Full inputs: logits/labels/weights [64, 1024, 1] f32. Output: scalar f32.

Data-parallel over batch: 8 cores x 8 batches. Pairwise term per batch
    T_b = sum_{i in pos} sum_{j in neg} relu((v_j + M) - v_i)
computed via a SORTED-BAND decomposition (the loss is invariant to per-batch
candidate order, and sorting is host-side layout prep):
  - host sorts pos ascending (a) and neg+M ascending (b) per batch;
  - pos ranks go to partitions in CH=5 chunks of 128 (rank 128c+p -> partition
    p, segment c of the tile);
  - for chunk c only the neg ranks in [W0_c, W0_c+need_c) can pair
    NON-trivially with the chunk (W0_c/hi_c from searchsorted): below-window
    j have b_j <= min_a (relu = 0), above-window j have b_j >= max_a (relu
    linear -> closed form);
  - ONE PE matmul per region builds psum[p, S_c+f] = s*(b[W0_c+f] - a[128c+p])
    directly: b rows (bf16 hi+lo, selector 1) add the b values, per-chunk
    "a rows" (selector -s*a hi+lo, b2 carrying the segment indicator)
    subtract a. s = 1/n_pairs(batch) folded into all values so batches can
    share accumulator columns;
  - the linear above-window remainder sum_p [SufB_c - a_p*C_c] rides in 2
    extra columns per chunk (C split 256q + r so the bf16 coefficients are
    exact); each term is provably >= 0 for real rows and < 0 for +BIG pads,
    so the same relu reduction handles it;
  - per batch ONE relu+row-sum instruction (ACT activation Relu accum_out or
    DVE tensor_scalar max0/add accum_out) over [128, 592] consumes the tile.
    Window splits are adaptive per batch (encoded in tile CONTENT, built on
    host); only the 592-col budget is fixed. P(batch exceeds budget) < 1e-4;
    such batches fall back to exact host compute.
  - batches are paired into [128, 1184] psum tiles (3 banks, 4 bank-aligned
    matmuls: 512|80|432|160) so PE runs long uninterrupted bursts.
  - BCE via ACT Softplus (same table set as Relu -> one ACT_TABLE_LOAD) +
    3 DVE ops; weights pre-scaled by 1/(B*N) on host.
Host: sorting/searchsorted/hi-lo encoding (layout prep), fallbacks, final
scalar in f64. Device does all O(N*band) reduction work + BCE.
"""
import sys
import numpy as np

sys.path.insert(0, "/opt/trn_rl_repo")

B, N = 64, 1024
N_CORES = 8
BLOC = B // N_CORES          # batches per core
CH = 5                       # pos-rank chunks of 128 (Pa <= 640 w.p. ~1-1e-15)
NXC = 2 * CH                 # X (remainder) columns per batch
CB = 592                     # per-batch tile budget (cols): NXC + windows
PAIRW = 2 * CB               # pair tile width (3 psum banks)
ROWS = 12                    # b hi/lo + 5x a hi/lo
SELW = BLOC * 128            # 1024 selector cols
B2W = BLOC * CB              # 4736 value cols
BFW = SELW + B2W             # combined bf16 tile width
MARGIN = 0.5
BIG = 16.0                   # pad sentinel; |b| <= ~7 << BIG

_CACHE = {}


def _patch_bass(bass):
    """Split multi-wait instructions (old walrus TPB_CTRL takes 1 wait)."""
    import json as _json
    if getattr(bass.Bass, "_wait_split_patched", False):
        return
    _orig = bass.Bass.to_json_bytes

    def _split(bir, limit=1):
        m = _json.loads(bir)
        for fn in m["functions"]:
            for bb in fn["blocks"]:
                out = []
                for i in bb.get("instructions", []):
                    si = i.get("sync_info") or {}
                    ow = si.get("on_wait") or []
                    if len(ow) > limit:
                        extra, keep = ow[:-limit], ow[-limit:]
                        for k, w in enumerate(extra):
                            out.append({
                                "debug": i.get("debug"), "engine": i["engine"],
                                "ins": [], "outs": [],
                                "name": i["name"] + f"_ws{k}",
                                "opcode": "NoOp",
                                "sync_info": {"on_wait": [w]},
                            })
                        si = dict(si)
                        si["on_wait"] = keep
                        i = dict(i)
                        i["sync_info"] = si
                    out.append(i)
                bb["instructions"] = out
        return _json.dumps(m).encode()

    bass.Bass.to_json_bytes = lambda self: _split(_orig(self))
    bass.Bass._wait_split_patched = True


def _build(bass, tile, mybir):
    f32 = mybir.dt.float32
    bf16 = mybir.dt.bfloat16
    Alu = mybir.AluOpType
    Act = mybir.ActivationFunctionType

    nc = bass.Bass()
    bf_d = nc.declare_dram_parameter("bf", [ROWS, BFW], bf16, isOutput=False)
    fv_d = nc.declare_dram_parameter("fv", [128, 192], f32, isOutput=False)
    outd_d = nc.declare_dram_parameter("outd", [128, 8], f32, isOutput=True)
    outa_d = nc.declare_dram_parameter("outa", [128, 8], f32, isOutput=True)

    with tile.TileContext(nc) as tc:
        with (
            tc.tile_pool(name="const", bufs=1) as const,
            tc.tile_pool(name="work", bufs=2) as work,
            tc.tile_pool(name="psum", bufs=2, space="PSUM") as psum,
        ):
            # engine-issued DMAs: the two HWDGE configs run in parallel on
            # the DVE/ACT sequencers instead of serially on Sync
            # bf (gates PE) configured first on the ACT sequencer so its 12
            # transfer chunks hit idle DMA queues; fv trails on SP
            bf = const.tile([ROWS, BFW], bf16)
            nc.scalar.dma_start(out=bf[:], in_=bf_d[:])
            fv = const.tile([128, 192], f32)
            nc.sync.dma_start(out=fv[:], in_=fv_d[:])

            acc_d = const.tile([128, 8], f32)
            acc_a = const.tile([128, 8], f32)
            nc.vector.memset(acc_d[:], 0.0)
            nc.vector.memset(acc_a[:], 0.0)

            # pair cols [0:1184) split 704 (DVE) / 480 (ACT) across two psum
            # tiles so the two consumers never read the SAME tile (the tile
            # framework serializes same-tile readers across engines).
            DW = 736

            def emit_mm(t):
                """Pair tiles for batches 2t, 2t+1; 4 bank-contained matmuls."""
                bcd = psum.tile([128, DW], f32, tag="bcd")
                bca = psum.tile([128, PAIRW - DW], f32, tag="bca")
                e, o = 2 * t, 2 * t + 1
                le = bf[:, 128 * e:128 * e + 128]
                lo_ = bf[:, 128 * o:128 * o + 128]
                obe = SELW + CB * e
                obo = SELW + CB * o
                nc.tensor.matmul(bcd[:, 0:512], le, bf[:, obe:obe + 512],
                                 start=True, stop=True)
                nc.tensor.matmul(bcd[:, 512:592], le, bf[:, obe + 512:obe + 592],
                                 start=True, stop=True)
                nc.tensor.matmul(bcd[:, 592:DW], lo_, bf[:, obo:obo + (DW - 592)],
                                 start=True, stop=True)
                nc.tensor.matmul(bca[:, 0:PAIRW - DW], lo_,
                                 bf[:, obo + (DW - 592):obo + CB],
                                 start=True, stop=True)
                return bcd, bca

            def ew_dve(bcd, col):
                scr = work.tile([128, DW], f32, tag="scr_d")
                nc.vector.tensor_scalar(
                    out=scr[:], in0=bcd[:], scalar1=0.0, scalar2=None,
                    op0=Alu.max, op1=Alu.add, accum_out=acc_d[:, col:col + 1])

            def ew_act(bca, col):
                scr = work.tile([128, PAIRW - DW], f32, tag="scr_a")
                nc.scalar.activation(
                    out=scr[:], in_=bca[:], func=Act.Relu,
                    accum_out=acc_a[:, col:col + 1])

            p0 = emit_mm(0)
            p1 = emit_mm(1)

            # BCE: sum w'*(softplus(v) - v*y) = sum w'*sp - sum wy*v with
            # wy = w'*y host-precomputed; softplus = Ln(Exp(v) + 1) via Ln's
            # bias (natural_log_exp table also holds Relu -> one table load)
            ex = work.tile([128, 64], f32)
            nc.scalar.activation(out=ex[:], in_=fv[:, 0:64], func=Act.Exp)
            sp = work.tile([128, 64], f32)
            nc.scalar.activation(out=sp[:], in_=ex[:], func=Act.Ln, bias=1.0)

            ew_dve(p0[0], 0)
            ew_act(p0[1], 0)

            b1 = work.tile([128, 64], f32)
            nc.vector.scalar_tensor_tensor(
                out=b1[:], in0=sp[:], scalar=1.0, op0=Alu.mult,
                op1=Alu.mult, in1=fv[:, 128:192], accum_out=acc_d[:, 4:5])
            b2s = work.tile([128, 64], f32)
            nc.vector.scalar_tensor_tensor(
                out=b2s[:], in0=fv[:, 0:64], scalar=-1.0, op0=Alu.mult,
                op1=Alu.mult, in1=fv[:, 64:128], accum_out=acc_d[:, 5:6])

            p2 = emit_mm(2)
            ew_dve(p1[0], 1)
            ew_act(p1[1], 1)
            p3 = emit_mm(3)
            ew_dve(p2[0], 2)
            ew_act(p2[1], 2)
            ew_dve(p3[0], 3)
            ew_act(p3[1], 3)

            nc.sync.dma_start(out=outd_d[:], in_=acc_d[:])
            nc.scalar.dma_start(out=outa_d[:], in_=acc_a[:])

    return nc


def _get_nc():
    if "nc" not in _CACHE:
        import concourse.bass as bass
        import concourse.tile as tile
        from concourse import mybir
        _patch_bass(bass)
        _CACHE["nc"] = _build(bass, tile, mybir)
    return _CACHE["nc"]


def _hi_lo(x):
    """f64 array -> (bf16 hi, bf16 lo) with hi+lo ~ x to ~2^-17 rel."""
    import ml_dtypes
    hi = x.astype(np.float32).astype(ml_dtypes.bfloat16)
    lo = (x - hi.astype(np.float64)).astype(np.float32).astype(ml_dtypes.bfloat16)
    return hi, lo


def _exact_mean(pos, neg):
    """Exact per-batch pairwise mean (f64); pos/neg sorted, neg has +M."""
    if len(pos) == 0 or len(neg) == 0:
        return 0.0
    dsum = 0.0
    # chunked to keep memory small
    for i0 in range(0, len(pos), 128):
        d = neg[None, :] - pos[i0:i0 + 128, None]
        dsum += float(np.maximum(d, 0.0).sum())
    return dsum / (len(pos) * len(neg))


def _prep_batch(vrow, yrow, selblk, b2blk):
    """Fill one batch's selector [ROWS,128] and value [ROWS,CB] blocks
    (f64, hi/lo split done by caller is NOT used -- we fill final f32 content
    here and caller casts). Returns (valid, fallback_mean_or_None)."""
    pos = np.sort(vrow[yrow == 1.0]).astype(np.float64)
    neg = np.sort(vrow[yrow == 0.0]).astype(np.float64) + MARGIN
    Pa, Nb = len(pos), len(neg)
    n_pairs = Pa * Nb
    if n_pairs == 0:
        return False, None            # invalid batch: zero content, mean 0
    if Pa > CH * 128:
        return True, _exact_mean(pos, neg)
    s = 1.0 / n_pairs

    W0s, needs = [], []
    for c in range(CH):
        lo_r = 128 * c
        if lo_r >= Pa:
            W0s.append(Nb)
            needs.append(0)
            continue
        hi_r = min(lo_r + 127, Pa - 1)
        w0 = int(np.searchsorted(neg, pos[lo_r], 'left'))
        hi = int(np.searchsorted(neg, pos[hi_r], 'right'))
        W0s.append(w0)
        needs.append(hi - w0)
    if sum(needs) + NXC > CB:
        return True, _exact_mean(pos, neg)

    negs = neg * s
    bval = np.zeros(CB, dtype=np.float64)     # b-row content (pre hi/lo)
    aind = np.zeros((CH, CB), dtype=np.float64)  # a-row indicator/coef rows
    avals = np.full((CH, 128), -s * BIG, dtype=np.float64)  # -s*a per chunk

    col = NXC
    for c in range(CH):
        w0, nd = W0s[c], needs[c]
        lo_r = 128 * c
        cnt = max(0, min(128, Pa - lo_r))
        if cnt > 0:
            avals[c, :cnt] = -s * pos[lo_r:lo_r + cnt]
        if nd > 0:
            bval[col:col + nd] = negs[w0:w0 + nd]
            aind[c, col:col + nd] = 1.0
        # remainder: C fully-active neg above the window
        E = w0 + nd
        C = Nb - E
        if C > 0:
            sufb = float(negs[E:].sum())
            q, r = C >> 8, C & 255
            xq, xr = 2 * c, 2 * c + 1
            if q > 0:
                bval[xq] = sufb * (256.0 * q / C)
                aind[c, xq] = 256.0 * q
            if r > 0:
                bval[xr] = sufb * (r / C)
                aind[c, xr] = r
        col += nd

    bhi, blo = _hi_lo(bval)
    b2blk[0, :] = bhi
    b2blk[1, :] = blo
    ahi, alo = _hi_lo(avals)
    for c in range(CH):
        b2blk[2 + 2 * c, :] = aind[c].astype(np.float32)
        b2blk[3 + 2 * c, :] = aind[c].astype(np.float32)
        selblk[2 + 2 * c, :] = ahi[c]
        selblk[3 + 2 * c, :] = alo[c]
    selblk[0, :] = 1.0
    selblk[1, :] = 1.0
    return True, None


def make_in_maps(v, y, w):
    import ml_dtypes
    in_maps, aux = [], []
    wsc = (w.astype(np.float64) / (B * N)).astype(np.float32)
    for core in range(N_CORES):
        sl = slice(core * BLOC, (core + 1) * BLOC)
        vb, yb, wb = v[sl], y[sl], wsc[sl]
        bft = np.zeros((ROWS, BFW), dtype=ml_dtypes.bfloat16)
        extra_mean = 0.0
        n_valid = 0
        for b in range(BLOC):
            selblk = np.zeros((ROWS, 128), dtype=ml_dtypes.bfloat16)
            b2blk = np.zeros((ROWS, CB), dtype=ml_dtypes.bfloat16)
            valid, fb = _prep_batch(vb[b], yb[b], selblk, b2blk)
            if valid:
                n_valid += 1
            if fb is not None:
                extra_mean += fb      # fallback: host-exact, zero content
            else:
                bft[:, 128 * b:128 * b + 128] = selblk
                bft[:, SELW + CB * b:SELW + CB * (b + 1)] = b2blk
        wy = (wb.astype(np.float64) * yb).astype(np.float32)
        fvt = np.concatenate(
            [vb.reshape(128, 64), wy.reshape(128, 64), wb.reshape(128, 64)],
            axis=1).astype(np.float32)
        in_maps.append({"bf": bft, "fv": np.ascontiguousarray(fvt)})
        aux.append({"extra_mean": extra_mean, "n_valid": n_valid})
    return in_maps, aux


def kernel(logits, labels, weights):
    from concourse.bass_utils import run_bass_kernel_spmd

    nc = _get_nc()
    v = np.ascontiguousarray(logits.reshape(B, N), dtype=np.float32)
    y = np.ascontiguousarray(labels.reshape(B, N), dtype=np.float32)
    w = np.ascontiguousarray(weights.reshape(B, N), dtype=np.float32)

    in_maps, aux = make_in_maps(v, y, w)
    res = run_bass_kernel_spmd(nc, in_maps, list(range(N_CORES)))

    mean_sum = 0.0
    bce_sum = 0.0
    valid_count = 0
    for c in range(N_CORES):
        od = np.asarray(res.results[c]["outd"]).astype(np.float64)
        oa = np.asarray(res.results[c]["outa"]).astype(np.float64)
        mean_sum += od[:, 0:4].sum() + oa[:, 0:4].sum()
        bce_sum += od[:, 4].sum() + od[:, 5].sum()
        mean_sum += aux[c]["extra_mean"]
        valid_count += aux[c]["n_valid"]
    rank_loss = mean_sum / valid_count if valid_count > 0 else 0.0
    return np.float32(bce_sum + rank_loss)


# revision 17
# speedup vs baseline: 1.7198x; 1.0381x over previous
"""Trainium2 Bass kernel for CombinedRankingLoss (BCE + pairwise margin ranking).

Full inputs: logits/labels/weights [64, 1024, 1] f32. Output: scalar f32.

Data-parallel over batch: 8 cores x 8 batches. Pairwise term per batch
    T_b = sum_{i in pos} sum_{j in neg} relu((v_j + M) - v_i)
computed via a SORTED-BAND decomposition (the loss is invariant to per-batch
candidate order, and sorting is host-side layout prep):
  - host sorts pos ascending (a) and neg+M ascending (b) per batch;
  - pos ranks go to partitions in CH=5 chunks of 128 (rank 128c+p -> partition
    p, segment c of the tile);
  - for chunk c only the neg ranks in [W0_c, W0_c+need_c) can pair
    NON-trivially with the chunk (W0_c/hi_c from searchsorted): below-window
    j have b_j <= min_a (relu = 0), above-window j have b_j >= max_a (relu
    linear -> closed form);
  - ONE PE matmul per region builds psum[p, S_c+f] = s*(b[W0_c+f] - a[128c+p])
    directly: b rows (bf16 hi+lo, selector 1) add the b values, per-chunk
    "a rows" (selector -s*a hi+lo, b2 carrying the segment indicator)
    subtract a. s = 1/n_pairs(batch) folded into all values so batches can
    share accumulator columns;
  - the linear above-window remainder sum_p [SufB_c - a_p*C_c] rides in 2
    extra columns per chunk (C split 256q + r so the bf16 coefficients are
    exact); each term is provably >= 0 for real rows and < 0 for +BIG pads,
    so the same relu reduction handles it;
  - per batch ONE relu+row-sum instruction (ACT activation Relu accum_out or
    DVE tensor_scalar max0/add accum_out) over [128, 592] consumes the tile.
    Window splits are adaptive per batch (encoded in tile CONTENT, built on
    host); only the 592-col budget is fixed. P(batch exceeds budget) < 1e-4;
    such batches fall back to exact host compute.
  - batches are paired into [128, 1184] psum tiles (3 banks, 4 bank-aligned
    matmuls: 512|80|432|160) so PE runs long uninterrupted bursts.
  - BCE via ACT Softplus (same table set as Relu -> one ACT_TABLE_LOAD) +
    3 DVE ops; weights pre-scaled by 1/(B*N) on host.
Host: sorting/searchsorted/hi-lo encoding (layout prep), fallbacks, final
scalar in f64. Device does all O(N*band) reduction work + BCE.
"""
import sys
import numpy as np

sys.path.insert(0, "/opt/trn_rl_repo")

B, N = 64, 1024
N_CORES = 8
BLOC = B // N_CORES          # batches per core
CH = 5                       # pos-rank chunks of 128 (Pa <= 640 w.p. ~1-1e-15)
NXC = 2 * CH                 # X (remainder) columns per batch
CB = 592                     # per-batch tile budget (cols): NXC + windows
PAIRW = 2 * CB               # pair tile width (3 psum banks)
ROWS = 12                    # b hi/lo + 5x a hi/lo
SELW = BLOC * 128            # 1024 selector cols
B2W = BLOC * CB              # 4736 value cols
BFW = SELW + B2W             # combined bf16 tile width
MARGIN = 0.5
BIG = 16.0                   # pad sentinel; |b| <= ~7 << BIG

_CACHE = {}


def _patch_bass(bass):
    """Split multi-wait instructions (old walrus TPB_CTRL takes 1 wait)."""
    import json as _json
    if getattr(bass.Bass, "_wait_split_patched", False):
        return
    _orig = bass.Bass.to_json_bytes

    def _split(bir, limit=1):
        m = _json.loads(bir)
        for fn in m["functions"]:
            for bb in fn["blocks"]:
                out = []
                for i in bb.get("instructions", []):
                    si = i.get("sync_info") or {}
                    ow = si.get("on_wait") or []
                    if len(ow) > limit:
                        extra, keep = ow[:-limit], ow[-limit:]
                        for k, w in enumerate(extra):
                            out.append({
                                "debug": i.get("debug"), "engine": i["engine"],
                                "ins": [], "outs": [],
                                "name": i["name"] + f"_ws{k}",
                                "opcode": "NoOp",
                                "sync_info": {"on_wait": [w]},
                            })
                        si = dict(si)
                        si["on_wait"] = keep
                        i = dict(i)
                        i["sync_info"] = si
                    out.append(i)
                bb["instructions"] = out
        return _json.dumps(m).encode()

    bass.Bass.to_json_bytes = lambda self: _split(_orig(self))
    bass.Bass._wait_split_patched = True


def _build(bass, tile, mybir):
    f32 = mybir.dt.float32
    bf16 = mybir.dt.bfloat16
    Alu = mybir.AluOpType
    Act = mybir.ActivationFunctionType

    nc = bass.Bass()
    W1 = SELW + 4 * CB                  # sel + b2 for batches 0-3
    W2 = 4 * CB                         # b2 for batches 4-7
    bf1_d = nc.declare_dram_parameter("bf1", [ROWS, W1], bf16, isOutput=False)
    bf2_d = nc.declare_dram_parameter("bf2", [ROWS, W2], bf16, isOutput=False)
    fv_d = nc.declare_dram_parameter("fv", [128, 192], f32, isOutput=False)
    outd_d = nc.declare_dram_parameter("outd", [128, 8], f32, isOutput=True)
    outa_d = nc.declare_dram_parameter("outa", [128, 8], f32, isOutput=True)

    with tile.TileContext(nc) as tc:
        with (
            tc.tile_pool(name="const", bufs=1) as const,
            tc.tile_pool(name="work", bufs=2) as work,
            tc.tile_pool(name="psum", bufs=2, space="PSUM") as psum,
        ):
            # engine-issued DMAs: the two HWDGE configs run in parallel on
            # the DVE/ACT sequencers instead of serially on Sync
            # bf1 (sel + first 4 batches, gates PE's first pairs) configured
            # first on the ACT sequencer; bf2/fv trail and stream in behind
            bf1 = const.tile([ROWS, W1], bf16)
            nc.scalar.dma_start(out=bf1[:], in_=bf1_d[:])
            bf2 = const.tile([ROWS, W2], bf16)
            nc.scalar.dma_start(out=bf2[:], in_=bf2_d[:])
            fv = const.tile([128, 192], f32)
            nc.sync.dma_start(out=fv[:], in_=fv_d[:])

            acc_d = const.tile([128, 8], f32)
            acc_a = const.tile([128, 8], f32)
            nc.vector.memset(acc_d[:], 0.0)
            nc.vector.memset(acc_a[:], 0.0)

            # pair cols [0:1184) split 704 (DVE) / 480 (ACT) across two psum
            # tiles so the two consumers never read the SAME tile (the tile
            # framework serializes same-tile readers across engines).
            DW = 736

            def emit_mm(t):
                """Pair tiles for batches 2t, 2t+1; 4 bank-contained matmuls."""
                bcd = psum.tile([128, DW], f32, tag="bcd")
                bca = psum.tile([128, PAIRW - DW], f32, tag="bca")
                e, o = 2 * t, 2 * t + 1
                le = bf1[:, 128 * e:128 * e + 128]
                lo_ = bf1[:, 128 * o:128 * o + 128]
                src = bf1 if t < 2 else bf2
                obe = (SELW if t < 2 else 0) + CB * (e % 4)
                obo = (SELW if t < 2 else 0) + CB * (o % 4)
                nc.tensor.matmul(bcd[:, 0:512], le, src[:, obe:obe + 512],
                                 start=True, stop=True)
                nc.tensor.matmul(bcd[:, 512:592], le, src[:, obe + 512:obe + 592],
                                 start=True, stop=True)
                nc.tensor.matmul(bcd[:, 592:DW], lo_, src[:, obo:obo + (DW - 592)],
                                 start=True, stop=True)
                nc.tensor.matmul(bca[:, 0:PAIRW - DW], lo_,
                                 src[:, obo + (DW - 592):obo + CB],
                                 start=True, stop=True)
                return bcd, bca

            def ew_dve(bcd, col):
                scr = work.tile([128, DW], f32, tag="scr_d")
                nc.vector.tensor_scalar(
                    out=scr[:], in0=bcd[:], scalar1=0.0, scalar2=None,
                    op0=Alu.max, op1=Alu.add, accum_out=acc_d[:, col:col + 1])

            def ew_act(bca, col):
                scr = work.tile([128, PAIRW - DW], f32, tag="scr_a")
                nc.scalar.activation(
                    out=scr[:], in_=bca[:], func=Act.Relu,
                    accum_out=acc_a[:, col:col + 1])

            p0 = emit_mm(0)
            p1 = emit_mm(1)

            # BCE: sum w'*(softplus(v) - v*y) = sum w'*sp - sum wy*v with
            # wy = w'*y host-precomputed; softplus = Ln(Exp(v) + 1) via Ln's
            # bias (natural_log_exp table also holds Relu -> one table load)
            ex = work.tile([128, 64], f32)
            nc.scalar.activation(out=ex[:], in_=fv[:, 0:64], func=Act.Exp)
            sp = work.tile([128, 64], f32)
            nc.scalar.activation(out=sp[:], in_=ex[:], func=Act.Ln, bias=1.0)

            ew_dve(p0[0], 0)
            ew_act(p0[1], 0)

            b1 = work.tile([128, 64], f32)
            nc.vector.scalar_tensor_tensor(
                out=b1[:], in0=sp[:], scalar=1.0, op0=Alu.mult,
                op1=Alu.mult, in1=fv[:, 128:192], accum_out=acc_d[:, 4:5])
            b2s = work.tile([128, 64], f32)
            nc.vector.scalar_tensor_tensor(
                out=b2s[:], in0=fv[:, 0:64], scalar=-1.0, op0=Alu.mult,
                op1=Alu.mult, in1=fv[:, 64:128], accum_out=acc_d[:, 5:6])

            p2 = emit_mm(2)
            ew_dve(p1[0], 1)
            ew_act(p1[1], 1)
            p3 = emit_mm(3)
            ew_dve(p2[0], 2)
            ew_act(p2[1], 2)
            ew_dve(p3[0], 3)
            ew_act(p3[1], 3)

            nc.sync.dma_start(out=outd_d[:], in_=acc_d[:])
            nc.scalar.dma_start(out=outa_d[:], in_=acc_a[:])

    return nc


def _get_nc():
    if "nc" not in _CACHE:
        import concourse.bass as bass
        import concourse.tile as tile
        from concourse import mybir
        _patch_bass(bass)
        _CACHE["nc"] = _build(bass, tile, mybir)
    return _CACHE["nc"]


def _hi_lo(x):
    """f64 array -> (bf16 hi, bf16 lo) with hi+lo ~ x to ~2^-17 rel."""
    import ml_dtypes
    hi = x.astype(np.float32).astype(ml_dtypes.bfloat16)
    lo = (x - hi.astype(np.float64)).astype(np.float32).astype(ml_dtypes.bfloat16)
    return hi, lo


def _exact_mean(pos, neg):
    """Exact per-batch pairwise mean (f64); pos/neg sorted, neg has +M."""
    if len(pos) == 0 or len(neg) == 0:
        return 0.0
    dsum = 0.0
    # chunked to keep memory small
    for i0 in range(0, len(pos), 128):
        d = neg[None, :] - pos[i0:i0 + 128, None]
        dsum += float(np.maximum(d, 0.0).sum())
    return dsum / (len(pos) * len(neg))


def _prep_batch(vrow, yrow, selblk, b2blk):
    """Fill one batch's selector [ROWS,128] and value [ROWS,CB] blocks
    (f64, hi/lo split done by caller is NOT used -- we fill final f32 content
    here and caller casts). Returns (valid, fallback_mean_or_None)."""
    pos = np.sort(vrow[yrow == 1.0]).astype(np.float64)
    neg = np.sort(vrow[yrow == 0.0]).astype(np.float64) + MARGIN
    Pa, Nb = len(pos), len(neg)
    n_pairs = Pa * Nb
    if n_pairs == 0:
        return False, None            # invalid batch: zero content, mean 0
    if Pa > CH * 128:
        return True, _exact_mean(pos, neg)
    s = 1.0 / n_pairs

    W0s, needs = [], []
    for c in range(CH):
        lo_r = 128 * c
        if lo_r >= Pa:
            W0s.append(Nb)
            needs.append(0)
            continue
        hi_r = min(lo_r + 127, Pa - 1)
        w0 = int(np.searchsorted(neg, pos[lo_r], 'left'))
        hi = int(np.searchsorted(neg, pos[hi_r], 'right'))
        W0s.append(w0)
        needs.append(hi - w0)
    if sum(needs) + NXC > CB:
        return True, _exact_mean(pos, neg)

    negs = neg * s
    bval = np.zeros(CB, dtype=np.float64)     # b-row content (pre hi/lo)
    aind = np.zeros((CH, CB), dtype=np.float64)  # a-row indicator/coef rows
    avals = np.full((CH, 128), -s * BIG, dtype=np.float64)  # -s*a per chunk

    col = NXC
    for c in range(CH):
        w0, nd = W0s[c], needs[c]
        lo_r = 128 * c
        cnt = max(0, min(128, Pa - lo_r))
        if cnt > 0:
            avals[c, :cnt] = -s * pos[lo_r:lo_r + cnt]
        if nd > 0:
            bval[col:col + nd] = negs[w0:w0 + nd]
            aind[c, col:col + nd] = 1.0
        # remainder: C fully-active neg above the window
        E = w0 + nd
        C = Nb - E
        if C > 0:
            sufb = float(negs[E:].sum())
            q, r = C >> 8, C & 255
            xq, xr = 2 * c, 2 * c + 1
            if q > 0:
                bval[xq] = sufb * (256.0 * q / C)
                aind[c, xq] = 256.0 * q
            if r > 0:
                bval[xr] = sufb * (r / C)
                aind[c, xr] = r
        col += nd

    bhi, blo = _hi_lo(bval)
    b2blk[0, :] = bhi
    b2blk[1, :] = blo
    ahi, alo = _hi_lo(avals)
    for c in range(CH):
        b2blk[2 + 2 * c, :] = aind[c].astype(np.float32)
        b2blk[3 + 2 * c, :] = aind[c].astype(np.float32)
        selblk[2 + 2 * c, :] = ahi[c]
        selblk[3 + 2 * c, :] = alo[c]
    selblk[0, :] = 1.0
    selblk[1, :] = 1.0
    return True, None


def make_in_maps(v, y, w):
    import ml_dtypes
    in_maps, aux = [], []
    wsc = (w.astype(np.float64) / (B * N)).astype(np.float32)
    for core in range(N_CORES):
        sl = slice(core * BLOC, (core + 1) * BLOC)
        vb, yb, wb = v[sl], y[sl], wsc[sl]
        bft = np.zeros((ROWS, BFW), dtype=ml_dtypes.bfloat16)
        extra_mean = 0.0
        n_valid = 0
        for b in range(BLOC):
            selblk = np.zeros((ROWS, 128), dtype=ml_dtypes.bfloat16)
            b2blk = np.zeros((ROWS, CB), dtype=ml_dtypes.bfloat16)
            valid, fb = _prep_batch(vb[b], yb[b], selblk, b2blk)
            if valid:
                n_valid += 1
            if fb is not None:
                extra_mean += fb      # fallback: host-exact, zero content
            else:
                bft[:, 128 * b:128 * b + 128] = selblk
                bft[:, SELW + CB * b:SELW + CB * (b + 1)] = b2blk
        wy = (wb.astype(np.float64) * yb).astype(np.float32)
        fvt = np.concatenate(
            [vb.reshape(128, 64), wy.reshape(128, 64), wb.reshape(128, 64)],
            axis=1).astype(np.float32)
        w1 = SELW + 4 * CB
        in_maps.append({"bf1": np.ascontiguousarray(bft[:, :w1]),
                        "bf2": np.ascontiguousarray(bft[:, w1:]),
                        "fv": np.ascontiguousarray(fvt)})
        aux.append({"extra_mean": extra_mean, "n_valid": n_valid})
    return in_maps, aux


def kernel(logits, labels, weights):
    from concourse.bass_utils import run_bass_kernel_spmd

    nc = _get_nc()
    v = np.ascontiguousarray(logits.reshape(B, N), dtype=np.float32)
    y = np.ascontiguousarray(labels.reshape(B, N), dtype=np.float32)
    w = np.ascontiguousarray(weights.reshape(B, N), dtype=np.float32)

    in_maps, aux = make_in_maps(v, y, w)
    res = run_bass_kernel_spmd(nc, in_maps, list(range(N_CORES)))

    mean_sum = 0.0
    bce_sum = 0.0
    valid_count = 0
    for c in range(N_CORES):
        od = np.asarray(res.results[c]["outd"]).astype(np.float64)
        oa = np.asarray(res.results[c]["outa"]).astype(np.float64)
        mean_sum += od[:, 0:4].sum() + oa[:, 0:4].sum()
        bce_sum += od[:, 4].sum() + od[:, 5].sum()
        mean_sum += aux[c]["extra_mean"]
        valid_count += aux[c]["n_valid"]
    rank_loss = mean_sum / valid_count if valid_count > 0 else 0.0
    return np.float32(bce_sum + rank_loss)
